# revision 5
# baseline (speedup 1.0000x reference)
"""Bipartite GATv2 (2 layers) Trainium2 Bass kernel, sharded over 8 NeuronCores by
destination node. Self-contained: host-side prep (edge sort/shard, one-hot builds)
+ Bass/Tile device program + gather/unshard.
"""
import sys

sys.path.insert(0, "/opt/trn_rl_repo")

from contextlib import ExitStack

import ml_dtypes
import numpy as np

import concourse.bacc as bacc
import concourse.bass as bass
import concourse.tile as tile
from concourse import mybir
from concourse.masks import make_identity

# problem constants (hardcoded per spec)
N_SRC, N_DST, E_TOT = 50000, 25000, 200000
D, H, EDGE_D = 128, 4, 32
HC = H * D  # 512
NEG_SLOPE = 0.2
LN_EPS = 1e-5
N_CORES = 8
DST_SHARD = N_DST // N_CORES  # 3125
BLK = 128
NB = (DST_SHARD + BLK - 1) // BLK  # 25 blocks of <=128 dsts
N_LAYERS = 2

BF16 = mybir.dt.bfloat16
F32 = mybir.dt.float32
I16 = mybir.dt.int16
AF = mybir.ActivationFunctionType
OP = mybir.AluOpType


def _bf(x):
    return np.asarray(x, dtype=np.float32).astype(ml_dtypes.bfloat16)


def _pack_idx16(idx, n_pad):
    """Pack int16 indices for dma_gather: [16, cols] wrap (j%16, j//16), tiled x8."""
    cols = n_pad // 16
    out = np.zeros((16, cols), np.int16)
    out.reshape(-1)[: len(idx)] = 0
    for base in range(0, len(idx), 16):
        chunk = idx[base : base + 16]
        out[: len(chunk), base // 16] = chunk
    return np.tile(out, (8, 1))


def _host_prep(send_rep, rec_rep, edge_rep, edge_index, params):
    """Shard + sort edges by dst; build all per-core device inputs."""
    src = np.asarray(edge_index[0])
    dst = np.asarray(edge_index[1])
    send_rep = np.asarray(send_rep, np.float32)
    rec_rep = np.asarray(rec_rep, np.float32)
    edge_rep = np.asarray(edge_rep, np.float32)

    cores = []
    tiles_per_block = np.zeros(NB, np.int64)  # cross-core max, per block
    for c in range(N_CORES):
        lo, hi = c * DST_SHARD, (c + 1) * DST_SHARD
        sel = np.nonzero((dst >= lo) & (dst < hi))[0]
        order = np.argsort(dst[sel], kind="stable")
        eids = sel[order]
        dloc = dst[eids] - lo  # 0..3124 sorted
        blk_of = dloc // BLK
        counts = np.bincount(blk_of, minlength=NB)
        tiles_per_block = np.maximum(tiles_per_block, (counts + 127) // 128)
        cores.append(dict(eids=eids, dloc=dloc, counts=counts))

    tiles_per_block = np.maximum(tiles_per_block, 1)
    blk_sizes = tiles_per_block * 128  # padded edges per block
    e_pad = int(blk_sizes.sum())
    blk_starts = np.concatenate([[0], np.cumsum(blk_sizes)]).astype(np.int64)

    # per-core padded arrays
    inputs = []
    u_counts = []
    for c in range(N_CORES):
        cc = cores[c]
        src_c = src[cc["eids"]]
        uniq, inv = np.unique(src_c, return_inverse=True)
        u_counts.append(len(uniq))
        cc["uniq"] = uniq
        cc["inv"] = inv
    u_max = ((max(u_counts) + 127) // 128) * 128
    assert u_max < 32768, "unique src count must fit int16"

    for c in range(N_CORES):
        cc = cores[c]
        eids, dloc, counts = cc["eids"], cc["dloc"], cc["counts"]
        src_loc = np.full(e_pad, 0, np.int64)  # pad edges gather row 0 (ignored)
        d_in_blk = np.full(e_pad, -1, np.int64)  # pad edges match no one-hot col
        e_sel = np.full(e_pad, -1, np.int64)  # -1 -> zero edge features
        pos = 0  # position in the core's sorted edge list
        for b in range(NB):
            n = int(counts[b])
            s0 = int(blk_starts[b])
            src_loc[s0 : s0 + n] = cc["inv"][pos : pos + n]
            d_in_blk[s0 : s0 + n] = dloc[pos : pos + n] - b * BLK
            e_sel[s0 : s0 + n] = eids[pos : pos + n]
            pos += n

        # one-hots (bf16 exact 0/1)
        iot = np.arange(BLK)
        S = (d_in_blk.reshape(-1, 128)[:, :, None] == iot[None, None, :])  # [T,128p,128c]
        S_host = np.ascontiguousarray(
            S.transpose(1, 0, 2).reshape(128, e_pad)
        ).astype(ml_dtypes.bfloat16)  # [128 p(e), T*128]
        Dm = (d_in_blk[None, :] == iot[:, None]).astype(ml_dtypes.bfloat16)  # [128 c, e_pad]

        edgeT = np.zeros((EDGE_D, e_pad), np.float32)
        m = e_sel >= 0
        edgeT[:, m] = edge_rep[e_sel[m]].T

        tab = np.zeros((u_max, D), np.float32)
        tab[: len(cc["uniq"])] = send_rep[cc["uniq"]]

        x0 = np.zeros((128, NB, D), np.float32)
        lo = c * DST_SHARD
        for b in range(NB):
            n = min(BLK, DST_SHARD - b * BLK)
            x0[:n, b, :] = rec_rep[lo + b * BLK : lo + b * BLK + n]

        im = dict(
            idxs=_pack_idx16(src_loc, e_pad),
            send_tab=tab.astype(ml_dtypes.bfloat16),
            edgeT=_bf(edgeT),
            S_h=S_host,
            D_h=np.ascontiguousarray(Dm),
            x0=x0,
        )
        inputs.append(im)

    # shared (replicated) weight-derived arrays
    shared = {}
    for l, p in enumerate(params):
        Wl = np.asarray(p["Wl"], np.float32)  # [512, 128]
        Wr = np.asarray(p["Wr"], np.float32)
        We = np.asarray(p["We"], np.float32)  # [512, 32]
        bl = np.asarray(p["bl"], np.float32)
        br = np.asarray(p["br"], np.float32)
        att = np.asarray(p["att"], np.float32)  # [4, 128]
        att_mask = np.zeros((128, 16), np.float32)
        for h in range(H):
            att_mask[:, 4 * h + h] = att[h]
        shared[f"WlT_{l}"] = _bf(Wl.T)  # [128, 512]
        shared[f"WrT_{l}"] = _bf(Wr.T)
        shared[f"WeT_{l}"] = _bf(We.T)  # [32, 512]
        shared[f"att_m_{l}"] = _bf(att_mask)
        shared[f"bias_lr_{l}"] = (bl + br).reshape(H, D).T.copy()  # [128, 4] f32
        shared[f"Wl_proj_{l}"] = _bf(0.25 * Wl.T)
        shared[f"bl_mat_{l}"] = _bf(0.25 * bl.reshape(H, D))  # [4, 128]
        shared[f"gbias_{l}"] = np.tile(np.asarray(p["bias"], np.float32)[None, :], (128, 1))
        shared[f"ln_g_{l}"] = np.tile(np.asarray(p["ln_g"], np.float32)[None, :], (128, 1))
        shared[f"ln_b_{l}"] = np.tile(np.asarray(p["ln_b"], np.float32)[None, :], (128, 1))

    meta = dict(e_pad=e_pad, u_max=u_max,
                tiles_per_block=[int(t) for t in tiles_per_block],
                blk_starts=[int(s) for s in blk_starts])
    return inputs, shared, meta, cores


def _build_program(meta):
    import os
    stage = int(os.environ.get("KSTAGE", "9"))
    e_pad = meta["e_pad"]
    u_max = meta["u_max"]
    tpb = meta["tiles_per_block"]
    bstart = meta["blk_starts"]

    nc = bacc.Bacc(None, target_bir_lowering=False)
    with tile.TileContext(nc) as tc, ExitStack() as ctx:
        dram = ctx.enter_context(tc.tile_pool(name="dram", bufs=1, space="DRAM"))

        def din(name, shape, dtype=BF16):
            return dram.tile(shape, dtype, kind="ExternalInput", name=name, uniquify=False)

        idxs_d = din("idxs", [128, e_pad // 16], I16)
        tab_d = din("send_tab", [u_max, D])
        edgeT_d = din("edgeT", [EDGE_D, e_pad])
        S_d = din("S_h", [128, e_pad])
        D_d = din("D_h", [128, e_pad])
        x0_d = din("x0", [128, NB, D], F32)
        Wp = {}
        for l in range(N_LAYERS):
            Wp[f"WlT_{l}"] = din(f"WlT_{l}", [128, HC])
            Wp[f"WrT_{l}"] = din(f"WrT_{l}", [128, HC])
            Wp[f"WeT_{l}"] = din(f"WeT_{l}", [EDGE_D, HC])
            Wp[f"att_m_{l}"] = din(f"att_m_{l}", [128, 16])
            Wp[f"bias_lr_{l}"] = din(f"bias_lr_{l}", [128, 4], F32)
            Wp[f"Wl_proj_{l}"] = din(f"Wl_proj_{l}", [128, HC])
            Wp[f"bl_mat_{l}"] = din(f"bl_mat_{l}", [4, D])
            Wp[f"gbias_{l}"] = din(f"gbias_{l}", [128, D], F32)
            Wp[f"ln_g_{l}"] = din(f"ln_g_{l}", [128, D], F32)
            Wp[f"ln_b_{l}"] = din(f"ln_b_{l}", [128, D], F32)
        out_d = dram.tile([128, NB, D], F32, kind="ExternalOutput", name="out", uniquify=False)

        # ---- pools ----
        const = ctx.enter_context(tc.tile_pool(name="const", bufs=1))
        big = ctx.enter_context(tc.tile_pool(name="bigsb", bufs=1))
        stream = ctx.enter_context(tc.tile_pool(name="stream", bufs=2))
        work = ctx.enter_context(tc.tile_pool(name="work", bufs=3))
        small = ctx.enter_context(tc.tile_pool(name="smallsb", bufs=4))
        ps_ut = ctx.enter_context(tc.tile_pool(name="ps_ut", bufs=4, space="PSUM"))
        ps_small = ctx.enter_context(tc.tile_pool(name="ps_small", bufs=3, space="PSUM"))
        ps_big = ctx.enter_context(tc.tile_pool(name="ps_big", bufs=1, space="PSUM"))

        ident_b = const.tile([128, 128], BF16)
        make_identity(nc, ident_b)
        ident_f = const.tile([128, 128], F32)
        make_identity(nc, ident_f)
        eps_col = const.tile([128, 1], F32)
        nc.vector.memset(eps_col, LN_EPS)

        # ---- load consts / weights to SBUF ----
        idx_sb = const.tile([128, e_pad // 16], I16)
        nc.sync.dma_start(out=idx_sb, in_=idxs_d[:])
        W = {}
        for k, dten in Wp.items():
            t = const.tile(list(dten.shape), dten.dtype, name=f"sb_{k}")
            nc.sync.dma_start(out=t, in_=dten[:])
            W[k] = t

        # ---- gathers (once, reused by both layers) ----
        if stage == -3:
            e_gather = 0
        GT = big.tile([128, 1, e_pad], BF16)   # [fin, e]
        GA = big.tile([128, e_pad // 128, 128], BF16)  # [e%128, e//128, fin]
        CH = int(os.environ.get("KGCH", "8192"))
        for s in range(0, e_pad if stage > -3 else 0, CH):
            n = min(CH, e_pad - s)
            if stage != -2:
                nc.gpsimd.dma_gather(
                    out_ap=GT[:, :, s : s + n], in_ap=tab_d[:], idxs_ap=idx_sb[:, s // 16 : (s + n) // 16],
                    num_idxs=n, num_idxs_reg=n, elem_size=D, transpose=True,
                    single_packet=False,
                )
            if stage != -1:
                nc.gpsimd.dma_gather(
                    out_ap=GA[:, s // 128 : (s + n) // 128, :], in_ap=tab_d[:],
                    idxs_ap=idx_sb[:, s // 16 : (s + n) // 16],
                    num_idxs=n, num_idxs_reg=n, elem_size=D,
                    single_packet=False,
                )

        # x_dst ping-pong
        xd = [big.tile([128, NB, D], F32, name=f"xd{i}") for i in range(2)]
        nc.sync.dma_start(out=xd[0][:], in_=x0_d[:])

        for l in range(N_LAYERS if stage >= 2 else 0):
            x_cur, x_nxt = xd[l % 2], xd[(l + 1) % 2]
            # ---- blocks ----
            for b in range(NB if stage >= 3 else 0):
                # xr rows for this block: xr[d, hc] = (x_dst @ Wr.T)
                xdT_ps = ps_small.tile([128, 128], F32, tag="sps", name="xdT_ps")
                nc.tensor.transpose(out=xdT_ps[:], in_=x_cur[:, b, :], identity=ident_f[:])
                xdT_sb = work.tile([128, 128], BF16, tag="xdT", name="xdT_sb")
                nc.vector.tensor_copy(out=xdT_sb[:], in_=xdT_ps[:])
                xr_ps = ps_big.tile([128, HC], F32, tag="big", name="xr_ps")
                nc.tensor.matmul(out=xr_ps[:], lhsT=xdT_sb[:], rhs=W[f"WrT_{l}"][:], start=True, stop=True)
                xr_blk = work.tile([128, HC], BF16, tag="xr", name="xr_blk")
                nc.scalar.copy(out=xr_blk[:], in_=xr_ps[:])
                T_b = tpb[b]
                s0 = bstart[b]
                nb_e = T_b * 128
                edgeT_blk = stream.tile([EDGE_D, nb_e], BF16, tag="edgeT", name="edgeT_blk")
                nc.sync.dma_start(out=edgeT_blk, in_=edgeT_d[:, s0 : s0 + nb_e])
                S_blk = stream.tile([128, nb_e], BF16, tag="S", name="S_blk")
                nc.sync.dma_start(out=S_blk, in_=S_d[:, s0 : s0 + nb_e])
                D_blk = stream.tile([128, nb_e], BF16, tag="D", name="D_blk")
                nc.sync.dma_start(out=D_blk, in_=D_d[:, s0 : s0 + nb_e])

                denom_ps = ps_small.tile([128, 4], F32, tag="sps", name="denom_ps")
                ex_blk = work.tile([128, T_b, 4], BF16, tag="ex", name="ex_blk")

                # pass 1: u, alpha, exp, denominators
                for c0 in range(0, T_b, 4):
                    ctiles = min(4, T_b - c0)
                    n = ctiles * 128
                    es = s0 + c0 * 128
                    uT = [ps_ut.tile([128, 512], F32, tag="ut", name=f"uT{h}") for h in range(H)]
                    for h in range(H):
                        nc.tensor.matmul(out=uT[h][:, :n], lhsT=W[f"WlT_{l}"][:, 128 * h : 128 * h + 128],
                                         rhs=GT[:, 0, es : es + n], start=True, stop=False)
                        nc.tensor.matmul(out=uT[h][:, :n], lhsT=W[f"WeT_{l}"][:, 128 * h : 128 * h + 128],
                                         rhs=edgeT_blk[:, c0 * 128 : c0 * 128 + n], start=False, stop=False)
                        nc.tensor.matmul(out=uT[h][:, :n], lhsT=xr_blk[:, 128 * h : 128 * h + 128],
                                         rhs=D_blk[:, c0 * 128 : c0 * 128 + n], start=False, stop=True)
                    sT = work.tile([128, H, 512], BF16, tag="sT", name="sT")
                    for h in range(H):
                        nc.scalar.activation(out=sT[:, h, :n], in_=uT[h][:, :n], func=AF.Prelu,
                                             bias=W[f"bias_lr_{l}"][:, h : h + 1], scale=1.0, alpha=NEG_SLOPE)
                    al_ps = ps_big.tile([4, 512], F32, tag="big", name="al_ps")
                    for h in range(H):
                        nc.tensor.matmul(out=al_ps[:, :n], lhsT=W[f"att_m_{l}"][:, 4 * h : 4 * h + 4],
                                         rhs=sT[:, h, :n], start=(h == 0), stop=(h == H - 1))
                    al_sb = small.tile([4, 512], F32, tag="al", name="al_sb")
                    nc.vector.tensor_copy(out=al_sb[:, :n], in_=al_ps[:, :n])
                    for t in range(ctiles):
                        at_ps = ps_small.tile([128, 4], F32, tag="sps", name="at_ps")
                        nc.tensor.transpose(out=at_ps[:], in_=al_sb[:, 128 * t : 128 * t + 128],
                                            identity=ident_f[:4, :4])
                        nc.scalar.activation(out=ex_blk[:, c0 + t, :], in_=at_ps[:], func=AF.Exp)
                        nc.tensor.matmul(out=denom_ps[:], lhsT=S_blk[:, (c0 + t) * 128 : (c0 + t) * 128 + 128],
                                         rhs=ex_blk[:, c0 + t, :], start=(c0 + t == 0), stop=(c0 + t == T_b - 1))

                if stage < 4:
                    continue
                # denominators -> reciprocal
                rden = small.tile([128, 4], F32, tag="rden", name="rden")
                nc.vector.tensor_scalar_add(rden[:], denom_ps[:], 1e-16)
                nc.vector.reciprocal(out=rden[:], in_=rden[:])
                rden_b = small.tile([128, 4], BF16, tag="rdenb", name="rden_b")
                nc.vector.tensor_copy(out=rden_b[:], in_=rden[:])
                tt = small.tile([128, 4], BF16, tag="tt", name="tt")
                nc.vector.tensor_tensor(out=tt[:], in0=denom_ps[:], in1=rden[:], op=OP.mult)
                ttT_ps = ps_small.tile([4, 128], BF16, tag="sps", name="ttT_ps")
                nc.tensor.transpose(out=ttT_ps[:], in_=tt[:], identity=ident_b[:])
                ttT = small.tile([4, 128], BF16, tag="ttT", name="ttT")
                nc.vector.tensor_copy(out=ttT[:], in_=ttT_ps[:])

                # pass 2: alpha-normalize, scaled scatter into z
                zT_ps = ps_big.tile([128, 512], F32, tag="big", name="zT_ps")
                for t in range(T_b):
                    e0 = t * 128
                    rdA_ps = ps_small.tile([128, 4], F32, tag="sps", name="rdA_ps")
                    nc.tensor.matmul(out=rdA_ps[:], lhsT=D_blk[:, e0 : e0 + 128], rhs=rden_b[:],
                                     start=True, stop=True)
                    an = small.tile([128, 4], BF16, tag="an", name="an")
                    nc.vector.tensor_tensor(out=an[:], in0=ex_blk[:, t, :], in1=rdA_ps[:], op=OP.mult)
                    sa = work.tile([128, 4, 128], BF16, tag="sa", name="sa")
                    s_bc = bass.AP(tensor=S_blk.tensor, offset=S_blk[:, e0 : e0 + 128].offset,
                                   ap=[S_blk.ap[0], [0, 4], [1, 128]])
                    a_bc = bass.AP(tensor=an.tensor, offset=an.offset,
                                   ap=[an.ap[0], an.ap[1], [0, 128]])
                    nc.vector.tensor_tensor(out=sa[:], in0=s_bc, in1=a_bc, op=OP.mult)
                    nc.tensor.matmul(out=zT_ps[:], lhsT=GA[:, (s0 + e0) // 128, :],
                                     rhs=sa.rearrange("p a b -> p (a b)"),
                                     start=(t == 0), stop=(t == T_b - 1))

                if stage < 5:
                    continue
                zT_sb = work.tile([128, 512], BF16, tag="zT", name="zT_sb")
                nc.scalar.copy(out=zT_sb[:], in_=zT_ps[:])
                out_ps = ps_big.tile([128, 128], F32, tag="big", name="out_ps")
                for h in range(H):
                    nc.tensor.matmul(out=out_ps[:], lhsT=zT_sb[:, 128 * h : 128 * h + 128],
                                     rhs=W[f"Wl_proj_{l}"][:, 128 * h : 128 * h + 128],
                                     start=(h == 0), stop=False)
                nc.tensor.matmul(out=out_ps[:], lhsT=ttT[:], rhs=W[f"bl_mat_{l}"][:],
                                 start=False, stop=True)

                # +bias +res, LayerNorm -> x_nxt[:, b, :]
                a_sb = work.tile([128, 128], F32, tag="ln_a", name="a_sb")
                nc.vector.tensor_tensor(out=a_sb[:], in0=out_ps[:], in1=W[f"gbias_{l}"][:], op=OP.add)
                nc.vector.tensor_tensor(out=a_sb[:], in0=a_sb[:], in1=x_cur[:, b, :], op=OP.add)
                stats = small.tile([128, 6], F32, tag="bst", name="stats")
                nc.vector.bn_stats(out=stats[:], in_=a_sb[:])
                mv = small.tile([128, 2], F32, tag="mv", name="mv")
                nc.vector.bn_aggr(out=mv[:], in_=stats[:])
                cen = work.tile([128, 128], F32, tag="cen", name="cen")
                nc.vector.tensor_scalar(out=cen[:], in0=a_sb[:], scalar1=mv[:, 0:1], scalar2=None,
                                        op0=OP.subtract)
                rstd = small.tile([128, 1], F32, tag="rstd", name="rstd")
                nc.scalar.activation(out=rstd[:], in_=mv[:, 1:2], func=AF.Ln, bias=eps_col[:])
                nc.scalar.activation(out=rstd[:], in_=rstd[:], func=AF.Exp, scale=-0.5)
                nc.vector.tensor_scalar(out=cen[:], in0=cen[:], scalar1=rstd[:, 0:1], scalar2=None,
                                        op0=OP.mult)
                nc.vector.tensor_tensor(out=cen[:], in0=cen[:], in1=W[f"ln_g_{l}"][:], op=OP.mult)
                nc.vector.tensor_tensor(out=x_nxt[:, b, :], in0=cen[:], in1=W[f"ln_b_{l}"][:], op=OP.add)

        nc.sync.dma_start(out=out_d[:], in_=xd[N_LAYERS % 2][:])

    nc.compile()
    return nc


_CACHE = {}
TRACE = False
LAST_RESULT = None


def kernel(send_rep, rec_rep, edge_rep, edge_index, params):
    inputs, shared, meta, cores = _host_prep(send_rep, rec_rep, edge_rep, edge_index, params)
    key = (meta["e_pad"], meta["u_max"], tuple(meta["tiles_per_block"]))
    if key not in _CACHE:
        _CACHE[key] = _build_program(meta)
    nc = _CACHE[key]

    in_maps = []
    for c in range(N_CORES):
        m = dict(inputs[c])
        m.update({k: np.asarray(v) for k, v in shared.items()})
        in_maps.append(m)

    from concourse.bass_utils import run_bass_kernel_spmd

    global LAST_RESULT
    res = run_bass_kernel_spmd(nc, in_maps, core_ids=list(range(N_CORES)), trace=TRACE)
    LAST_RESULT = res

    out = np.zeros((N_DST, D), np.float32)
    for c in range(N_CORES):
        arr = res.results[c]["out"]  # [128, NB, D]
        lo = c * DST_SHARD
        for b in range(NB):
            n = min(BLK, DST_SHARD - b * BLK)
            out[lo + b * BLK : lo + b * BLK + n] = arr[:n, b, :]
    return out
